# revision 32
# baseline (speedup 1.0000x reference)
"""Trainium2 Bass kernel for the CriticalField PDE step.

Computes one explicit step of a coupled magnitude/phase field update on a
4096x4096 grid with circular boundary conditions:

    mag_lap   = 4-neighbor circular Laplacian of magnitude
    phase_lap = 4-neighbor circular Laplacian of phase
    d_mag     = tension*mag_lap - damping*mag - nonlinearity*mag^3
    d_phase   = tension*phase_lap + COUPLING*sin(up(phase) - phase)
    out[0]    = clip(mag + DT*d_mag, -2, 2)
    out[1]    = clip(phase + DT*d_phase, 0, 2*pi)

Numerical shortcuts (verified against the fp32 reference, max rel err
~4.3e-3 vs the 2e-2 budget):
  - The sin coupling term moves phase by at most DT*COUPLING = 7.5e-4 rad,
    16x below the u8 output quantization step (2pi/255 = 0.0246 rad), so it
    is dropped entirely.
  - Without sin, the phase update is a convex combination of values in
    [0, 2pi] (A2 + 4B = 1, all coefficients >= 0), so the clip is a no-op.
    The phase output scale is shrunk by 2e-3 so fp16 rounding of weights and
    inputs can never push the pre-convert value above 255.45; the drain is
    then one Activation-engine Copy (f32 PSUM -> u8, round-to-nearest).
  - mag in: fp16; phase in: fp16; mag out: int8 scale 63.5; phase out: uint8.

Engine split (scalar_tensor_tensor has NO DVE perf modes, so only
tensor_tensor / tensor_scalar shapes are used on DVE):
  PE   (6 matmuls per 512-col block):
        pm = w_m_tri@mg_c + w_B@lr_m + (-SM)@c3
        pp = w_p_tri@ph_c + w_B@ph_l + w_B@ph_r
  DVE:  lr_m = l+r and c3 = c2*m (tensor_tensor, 2x mode, 1024-wide pairs),
        mag clip-drain ts(pm, 127, -127, min, max) -> i8 (1x, PSUM read)
  Act:  c2 = Square(sqrt(Cc)*m) (1024-wide, one pair AHEAD of use),
        phase drain Copy -> u8
  Pool: only the 4 tiny column-halo copies per tile.

Scheduling: one flat stream of 1024-col pairs across the overflow block and
all 4 tiles (no per-tile barriers). Per pair: c2 for the NEXT pair (Act),
DVE producers, then drains of the PREVIOUS pair, then this pair's matmuls.
Producing c2 a pair ahead and draining with a full-pair lag keeps every
in-order engine queue free of same-block PE round-trips; draining before
the matmuls means the PSUM ring slot a matmul reuses (4+4 banks) was
already drained upstream in the same queue. Stores go out per half-tile as
soon as the half is drained.

Sharding: rows split across 8 NeuronCores; each core gets 504 rows as 4
tiles of 128 partitions (126 valid rows each) plus 1/8 of the 64 leftover
rows as a column-split overflow block. Row halos are materialized host-side;
column halos are produced on-device by copying the wrap columns.
"""

import numpy as np

SIZE = 4096
NCORES = 8
TILE_VALID = 126
NTILES = 4
MAIN_ROWS = TILE_VALID * NTILES          # 504 rows per core via main tiles
OVF_ROWS = SIZE - MAIN_ROWS * NCORES     # 64 leftover rows (4032..4095)
OVF_COLS = SIZE // NCORES                # 512 columns of overflow per core
DT = 0.05
COUPLING = 0.015
TWO_PI = 2.0 * np.pi
SM = 63.5                                # mag output quant scale
SP_MARGIN = 1.0 - 2e-3                   # keep pre-convert phase < 255.45
SP_IN = 255.0 / TWO_PI

_PROG_CACHE: dict = {}
_WEIGHTS_CACHE: dict = {}


def _make_weights(damping, tension):
    """lhsT weight matrices for nc.tensor.matmul (out = lhsT.T @ rhs).

    Five 128x128 blocks: [w_m_tri | w_p_tri | w_Bm | w_Bp | w_negSM].
    Tridiagonal blocks carry the center coefficient and up/down-neighbor
    coupling; diagonal blocks apply the left+right sums and the cubic term.
    Output quantization scales (SM, SPO) are folded in so PSUM holds the
    finished pre-convert value.
    """
    key = (float(damping), float(tension))
    if key in _WEIGHTS_CACHE:
        return _WEIGHTS_CACHE[key]
    A = 1.0 - 4.0 * DT * tension - DT * damping
    A2 = 1.0 - 4.0 * DT * tension
    B = DT * tension
    SPO = SP_IN * SP_MARGIN
    idx = np.arange(127)
    w_ud = np.zeros((128, 128), np.float32)
    w_ud[idx, idx + 1] = 1.0      # k = m-1 -> up neighbor
    w_ud[idx + 1, idx] = 1.0      # k = m+1 -> down neighbor
    eye = np.eye(128, dtype=np.float32)
    w_m_tri = SM * (B * w_ud + A * eye)
    w_p_tri = SPO * (B * w_ud + A2 * eye)
    w_Bm = SM * B * eye
    w_Bp = SPO * B * eye
    w_negSM = -SM * eye           # applied to c3 = Cc*m^3
    w_all = np.concatenate(
        [w_m_tri, w_p_tri, w_Bm, w_Bp, w_negSM], axis=1).astype(np.float16)

    # No-clip safety for the phase drain: with all-(almost 2pi) inputs the
    # PSUM value must stay under 255.45 despite fp16 rounding of weights
    # and of the host-converted phase.
    ph_max = np.float32(np.float16(TWO_PI))  # host fp16 may round 2pi UP
    wA2 = np.float32(np.float16(SPO * A2))
    wB = np.float32(np.float16(SPO * B))
    pp_max = float(ph_max * (wA2 + 4.0 * wB))
    assert pp_max < 255.45, pp_max

    w = {"w_all": np.ascontiguousarray(w_all), "SPO": SPO}
    _WEIGHTS_CACHE[key] = w
    return w


def _build_program(Cc, repeat=1, mode="full", hw_loop=False):
    import concourse.bass as bass
    import concourse.bacc as bacc
    import concourse.tile as tile
    from concourse import mybir

    f16 = mybir.dt.float16
    f32 = mybir.dt.float32
    u8 = mybir.dt.uint8
    i8 = mybir.dt.int8
    Act = mybir.ActivationFunctionType
    Alu = mybir.AluOpType

    sqrtCc = float(np.sqrt(Cc))

    nc = bacc.Bacc(trn_type="TRN2", target_bir_lowering=False, debug=False)

    mag_slab = nc.dram_tensor("mag_slab", [MAIN_ROWS + 2, SIZE], f16,
                              kind="ExternalInput").ap()
    ph_slab = nc.dram_tensor("ph_slab", [MAIN_ROWS + 2, SIZE], f16,
                             kind="ExternalInput").ap()
    mag_ovf = nc.dram_tensor("mag_ovf", [OVF_ROWS + 2, OVF_COLS + 2], f16,
                             kind="ExternalInput").ap()
    ph_ovf = nc.dram_tensor("ph_ovf", [OVF_ROWS + 2, OVF_COLS + 2], f16,
                            kind="ExternalInput").ap()
    w_all_d = nc.dram_tensor("w_all", [128, 640], f16, kind="ExternalInput").ap()
    out_mag = nc.dram_tensor("out_mag", [MAIN_ROWS, SIZE], i8,
                             kind="ExternalOutput").ap()
    out_ph = nc.dram_tensor("out_ph", [MAIN_ROWS, SIZE], u8,
                            kind="ExternalOutput").ap()
    out_ovf_mag = nc.dram_tensor("out_ovf_mag", [OVF_ROWS, OVF_COLS], i8,
                                 kind="ExternalOutput").ap()
    out_ovf_ph = nc.dram_tensor("out_ovf_ph", [OVF_ROWS, OVF_COLS], u8,
                                kind="ExternalOutput").ap()

    with tile.TileContext(nc) as tc:
        with (
            tc.tile_pool(name="wts", bufs=1) as wpool,
            tc.tile_pool(name="inp", bufs=3) as inp,
            tc.tile_pool(name="outp", bufs=2) as outp,
            tc.tile_pool(name="sml", bufs=4) as sml,
            tc.tile_pool(name="psm", bufs=4, space="PSUM") as psm,
            tc.tile_pool(name="psp", bufs=4, space="PSUM") as psp,
        ):
            w_all = wpool.tile([128, 640], f16, tag="w_all")
            nc.sync.dma_start(w_all[:, :], w_all_d[:, :])

            def wslice(P):
                return (w_all[0:P, 0:P], w_all[0:P, 128:128 + P],
                        w_all[0:P, 256:256 + P], w_all[0:P, 384:384 + P],
                        w_all[0:P, 512:512 + P])

            def load_tile(ti, halos=True):
                # phase first: the Pool lr_p precompute depends only on it.
                t0 = TILE_VALID * ti
                ph = inp.tile([128, SIZE + 2], f16, tag="ph")
                nc.sync.dma_start(ph[:, 1:1 + SIZE], ph_slab[t0:t0 + 128, :])
                mg = inp.tile([128, SIZE + 2], f16, tag="mg")
                nc.sync.dma_start(mg[:, 1:1 + SIZE], mag_slab[t0:t0 + 128, :])
                if halos:
                    # Circular column halos (diagnostic modes only; the full
                    # path patches edge columns instead).
                    nc.gpsimd.tensor_copy(mg[:, 0:1], mg[:, SIZE:SIZE + 1])
                    nc.gpsimd.tensor_copy(mg[:, SIZE + 1:SIZE + 2], mg[:, 1:2])
                    nc.gpsimd.tensor_copy(ph[:, 0:1], ph[:, SIZE:SIZE + 1])
                    nc.gpsimd.tensor_copy(ph[:, SIZE + 1:SIZE + 2], ph[:, 1:2])
                return mg, ph

            def emit_block_diag(mg, ph, om, op_, P, ncols):
                """Diagnostic mode ladder (timing-only, garbage outputs):
                dma = loads/stores; pe = +matmuls; pedve = +DVE ops+clip."""
                if mode == "dma":
                    nc.vector.memset(om[0:P, 0:ncols], 0)
                    nc.vector.memset(op_[0:P, 0:ncols], 0)
                    return
                do_dve = mode == "pedve"
                if mode == "pe":
                    nc.vector.memset(om[0:P, 0:ncols], 0)
                nc.vector.memset(op_[0:P, 0:ncols], 0)
                w_m_tri, w_p_tri, w_Bm, w_Bp, w_negSM = wslice(P)
                pending = []

                def compute(j):
                    cw = min(512, ncols - j)
                    mg_c = mg[0:P, 1 + j:1 + j + cw]
                    mg_l = mg[0:P, j:j + cw]
                    mg_r = mg[0:P, 2 + j:2 + j + cw]
                    ph_c = ph[0:P, 1 + j:1 + j + cw]
                    ph_l = ph[0:P, j:j + cw]
                    ph_r = ph[0:P, 2 + j:2 + j + cw]
                    pm = psm.tile([P, cw], f32, tag="pm")
                    if do_dve:
                        lr_m = sml.tile([P, cw], f16, tag="lr_m")
                        nc.vector.tensor_tensor(lr_m[:, :], mg_l, mg_r, Alu.add)
                        c3 = sml.tile([P, cw], f16, tag="c3")
                        nc.vector.tensor_tensor(c3[:, :], mg_c, mg_c, Alu.mult)
                        nc.tensor.matmul(pm[:, :], w_m_tri, mg_c,
                                         start=True, stop=False)
                        nc.tensor.matmul(pm[:, :], w_Bm, lr_m[:, :],
                                         start=False, stop=False)
                        nc.tensor.matmul(pm[:, :], w_negSM, c3[:, :],
                                         start=False, stop=True)
                    else:
                        nc.tensor.matmul(pm[:, :], w_m_tri, mg_c,
                                         start=True, stop=False)
                        nc.tensor.matmul(pm[:, :], w_Bm, mg_l,
                                         start=False, stop=False)
                        nc.tensor.matmul(pm[:, :], w_Bm, mg_r,
                                         start=False, stop=True)
                    pp = psp.tile([P, cw], f32, tag="pp")
                    nc.tensor.matmul(pp[:, :], w_p_tri, ph_c,
                                     start=True, stop=False)
                    nc.tensor.matmul(pp[:, :], w_Bp, ph_l,
                                     start=False, stop=False)
                    nc.tensor.matmul(pp[:, :], w_Bp, ph_r,
                                     start=False, stop=True)
                    return j, cw, pm, pp

                for j in range(0, ncols, 512):
                    pending.append(compute(j))
                    if do_dve and len(pending) > 2:
                        jd, cw, pm, pp = pending.pop(0)
                        nc.vector.tensor_scalar(
                            om[0:P, jd:jd + cw], pm[:, :],
                            127.0, -127.0, Alu.min, Alu.max)
                if do_dve:
                    for jd, cw, pm, pp in pending:
                        nc.vector.tensor_scalar(
                            om[0:P, jd:jd + cw], pm[:, :],
                            127.0, -127.0, Alu.min, Alu.max)

            def emit_rep_diag():
              P = OVF_ROWS + 2
              mg = inp.tile([P, OVF_COLS + 2], f16, tag="mgo")
              nc.sync.dma_start(mg[:, :], mag_ovf[:, :])
              ph = inp.tile([P, OVF_COLS + 2], f16, tag="pho")
              nc.sync.dma_start(ph[:, :], ph_ovf[:, :])
              om = outp.tile([P, OVF_COLS], i8, tag="omo")
              op_ = outp.tile([P, OVF_COLS], u8, tag="opo")
              emit_block_diag(mg, ph, om, op_, P, OVF_COLS)
              nc.sync.dma_start(out_ovf_mag[:, :], om[1:P - 1, :])
              nc.sync.dma_start(out_ovf_ph[:, :], op_[1:P - 1, :])
              cur = load_tile(0)
              for ti in range(NTILES):
                if ti + 1 < NTILES:
                    nxt = load_tile(ti + 1)
                t0 = TILE_VALID * ti
                om = outp.tile([128, SIZE], i8, tag="om")
                op_ = outp.tile([128, SIZE], u8, tag="op")
                emit_block_diag(cur[0], cur[1], om, op_, 128, SIZE)
                nc.sync.dma_start(out_mag[t0:t0 + TILE_VALID, :],
                                  om[1:127, :])
                nc.sync.dma_start(out_ph[t0:t0 + TILE_VALID, :],
                                  op_[1:127, :])
                if ti + 1 < NTILES:
                    cur = nxt

            def emit_rep():
                """Full mode: one flat stream of 512-col blocks across the
                overflow block and all 4 main tiles. c2 is produced one
                stream-item ahead (Act), drains lag two items (DVE clip +
                Act copy), stores go out per half-tile as soon as the half
                is drained. This keeps every in-order engine queue free of
                same-block PE round-trips, including across tile boundaries.
                """
                items = []          # (tctx, j)

                P_OVF = OVF_ROWS + 2
                mgo = inp.tile([P_OVF, OVF_COLS + 2], f16, tag="mgo")
                nc.sync.dma_start(mgo[:, :], mag_ovf[:, :])
                pho = inp.tile([P_OVF, OVF_COLS + 2], f16, tag="pho")
                nc.sync.dma_start(pho[:, :], ph_ovf[:, :])
                omo = outp.tile([P_OVF, OVF_COLS], i8, tag="omo")
                opo = outp.tile([P_OVF, OVF_COLS], u8, tag="opo")

                def ovf_store():
                    nc.sync.dma_start(out_ovf_mag[:, :], omo[1:P_OVF - 1, :])
                    nc.sync.dma_start(out_ovf_ph[:, :], opo[1:P_OVF - 1, :])

                ovf_ctx = {"mg": mgo, "ph": pho, "om": omo, "op": opo,
                           "P": P_OVF, "drained": 0, "W": OVF_COLS,
                           "has_halo": True, "stores": {1: ovf_store}}
                items.append((ovf_ctx, 0))

                def mk_tile_ctx(ti, loaded):
                    t0 = TILE_VALID * ti
                    om = outp.tile([128, SIZE], i8, tag="om")
                    op_ = outp.tile([128, SIZE], u8, tag="op")
                    HALF = SIZE // 2

                    def store_half(lo):
                        def go():
                            nc.sync.dma_start(
                                out_mag[t0:t0 + TILE_VALID, lo:lo + HALF],
                                om[1:127, lo:lo + HALF])
                            nc.sync.dma_start(
                                out_ph[t0:t0 + TILE_VALID, lo:lo + HALF],
                                op_[1:127, lo:lo + HALF])
                        return go

                    return {"mg": loaded[0], "ph": loaded[1], "om": om,
                            "op": op_, "P": 128, "drained": 0, "W": SIZE,
                            "has_halo": False,
                            "stores": {4: store_half(0), 8: store_half(HALF)}}

                tiles_loaded = [load_tile(0)]

                def emit_c2(tctx, j0, w):
                    P = tctx["P"]
                    mg_c = tctx["mg"][0:P, 1 + j0:1 + j0 + w]
                    c2 = sml.tile([P, w], f16, tag="c2")
                    nc.scalar.activation(c2[:, :], mg_c, Act.Square,
                                         bias=0.0, scale=sqrtCc)
                    return c2

                def produce(tctx, j0, w, c2):
                    # DVE pair-wide producers for blocks [j0, j0+w)
                    P = tctx["P"]
                    mg = tctx["mg"]
                    mg_c = mg[0:P, 1 + j0:1 + j0 + w]
                    lr_m = sml.tile([P, w], f16, tag="lr_m")
                    nc.vector.tensor_tensor(
                        lr_m[:, :], mg[0:P, j0:j0 + w],
                        mg[0:P, 2 + j0:2 + j0 + w], Alu.add)
                    c3 = sml.tile([P, w], f16, tag="c3")
                    nc.vector.tensor_tensor(c3[:, :], c2[:, :], mg_c, Alu.mult)
                    return lr_m, c3

                def mms(tctx, j0, w, lr_m, c3):
                    # One 2-bank PSUM tile per field per pair; each 512-col
                    # half is its own accumulation group. The pair is later
                    # drained by ONE wide op per field.
                    P = tctx["P"]
                    mg, ph = tctx["mg"], tctx["ph"]
                    w_m_tri, w_p_tri, w_Bm, w_Bp, w_negSM = wslice(P)
                    pm2 = psm.tile([P, w], f32, tag="pm2", bufs=2)
                    pp2 = psp.tile([P, w], f32, tag="pp2", bufs=2)
                    for h in range(0, w, 512):
                        j = j0 + h
                        mg_c = mg[0:P, 1 + j:1 + j + 512]
                        ph_c = ph[0:P, 1 + j:1 + j + 512]
                        pm = pm2[0:P, h:h + 512]
                        nc.tensor.matmul(pm, w_m_tri, mg_c,
                                         start=True, stop=False)
                        nc.tensor.matmul(pm, w_Bm, lr_m[0:P, h:h + 512],
                                         start=False, stop=False)
                        nc.tensor.matmul(pm, w_negSM, c3[0:P, h:h + 512],
                                         start=False, stop=True)
                        pp = pp2[0:P, h:h + 512]
                        nc.tensor.matmul(pp, w_p_tri, ph_c,
                                         start=True, stop=False)
                        nc.tensor.matmul(pp, w_Bp, ph[0:P, j:j + 512],
                                         start=False, stop=False)
                        nc.tensor.matmul(pp, w_Bp,
                                         ph[0:P, 2 + j:2 + j + 512],
                                         start=False, stop=True)
                    return [(tctx, j0, w, pm2, pp2)]

                def drain_item(it):
                    tctx, j0, w, pm2, pp2 = it
                    P = tctx["P"]
                    nc.vector.tensor_scalar(
                        tctx["om"][0:P, j0:j0 + w], pm2[0:P, 0:w],
                        127.0, -127.0, Alu.min, Alu.max)
                    nc.scalar.activation(
                        tctx["op"][0:P, j0:j0 + w], pp2[0:P, 0:w], Act.Copy,
                        bias=0.0, scale=1.0)
                    tctx["drained"] += w // 512
                    for thr in list(tctx["stores"]):
                        if tctx["drained"] >= thr:
                            tctx["stores"].pop(thr)()

                # Stream of pair-items: the ovf block (one 512 half) then 4
                # pairs (1024 cols) per tile. Per item: c2 for the NEXT item
                # (Act), DVE producers, then drains of the PREVIOUS item,
                # then this item's matmuls. Drains-before-matmuls means the
                # PSUM ring slot a matmul reuses was already drained
                # in-stream, so the PE never waits on a drain.
                stream = [("ovf", 0, 512)] + [
                    (ti, jp * 1024, 1024) for ti in range(NTILES)
                    for jp in range(4)]
                ctxs = {"ovf": ovf_ctx}
                pending = []
                c2 = emit_c2(ovf_ctx, 0, 512)
                for idx, (ti, j0, w) in enumerate(stream):
                    if ti != "ovf" and j0 == 0:
                        ctxs[ti] = mk_tile_ctx(ti, tiles_loaded[ti])
                        if ti + 1 < NTILES:
                            tiles_loaded.append(load_tile(ti + 1))
                    tctx = ctxs[ti]
                    if idx + 1 < len(stream):
                        nti, nj0, nw = stream[idx + 1]
                        if nti in ctxs:
                            c2_next = emit_c2(ctxs[nti], nj0, nw)
                        else:
                            nmg = tiles_loaded[nti][0]
                            c2_next = emit_c2(
                                {"P": 128, "mg": nmg}, nj0, nw)
                    else:
                        c2_next = None
                    lr_m, c3 = produce(tctx, j0, w, c2)
                    while pending:
                        drain_item(pending.pop(0))
                    pending.extend(mms(tctx, j0, w, lr_m, c3))
                    c2 = c2_next
                for it in pending:
                    drain_item(it)

            rep_fn = emit_rep if mode == "full" else emit_rep_diag
            if hw_loop and repeat > 1:
                with tc.For_i(0, repeat, 1):
                    rep_fn()
            else:
                for _rep in range(repeat):
                    rep_fn()

    nc.compile()
    return nc


def _get_program(damping, tension, nonlinearity, repeat=1, mode="full",
                 hw_loop=False):
    key = (damping, tension, nonlinearity, repeat, mode, hw_loop)
    if key not in _PROG_CACHE:
        Cc = DT * nonlinearity
        _PROG_CACHE[key] = _build_program(Cc, repeat, mode, hw_loop)
    return _PROG_CACHE[key]


def _make_in_maps(mag, ph, damping=0.05, tension=1.5):
    """Per-core input dicts: fp16 mag and phase, circular row halos."""
    w = _make_weights(damping, tension)
    mag16 = mag.astype(np.float16)
    ph16 = ph.astype(np.float16)
    cols = np.arange(-1, SIZE + 1) % SIZE
    ovf_rows = np.arange(MAIN_ROWS * NCORES - 1, SIZE + 1) % SIZE
    mag_ovf_full = mag16[np.ix_(ovf_rows, cols)]
    ph_ovf_full = ph16[np.ix_(ovf_rows, cols)]
    in_maps = []
    for m in range(NCORES):
        rows = np.arange(MAIN_ROWS * m - 1, MAIN_ROWS * (m + 1) + 1) % SIZE
        c0 = OVF_COLS * m
        in_maps.append({
            "mag_slab": np.ascontiguousarray(mag16[rows, :]),
            "ph_slab": np.ascontiguousarray(ph16[rows, :]),
            "mag_ovf": np.ascontiguousarray(mag_ovf_full[:, c0:c0 + OVF_COLS + 2]),
            "ph_ovf": np.ascontiguousarray(ph_ovf_full[:, c0:c0 + OVF_COLS + 2]),
            "w_all": w["w_all"],
        })
    return in_maps


def _assemble(results, SPO):
    out = np.empty((1, 2, SIZE, SIZE), np.float32)
    for m in range(NCORES):
        r = results[m]
        r0, r1 = MAIN_ROWS * m, MAIN_ROWS * (m + 1)
        out[0, 0, r0:r1, :] = r["out_mag"].astype(np.float32) / SM
        out[0, 1, r0:r1, :] = r["out_ph"].astype(np.float32) / SPO
        c0, c1 = OVF_COLS * m, OVF_COLS * (m + 1)
        out[0, 0, MAIN_ROWS * NCORES:, c0:c1] = \
            r["out_ovf_mag"].astype(np.float32) / SM
        out[0, 1, MAIN_ROWS * NCORES:, c0:c1] = \
            r["out_ovf_ph"].astype(np.float32) / SPO
    return out


def kernel(magnitude, phase, damping, tension, nonlinearity):
    from concourse.bass_utils import run_bass_kernel_spmd

    mag = np.asarray(magnitude, dtype=np.float32).reshape(SIZE, SIZE)
    ph = np.asarray(phase, dtype=np.float32).reshape(SIZE, SIZE)
    d = float(np.asarray(damping))
    tn = float(np.asarray(tension))
    nl = float(np.asarray(nonlinearity))

    nc = _get_program(d, tn, nl)
    in_maps = _make_in_maps(mag, ph, d, tn)
    res = run_bass_kernel_spmd(nc, in_maps, core_ids=list(range(NCORES)))
    w = _make_weights(d, tn)
    return _assemble(res.results, w["SPO"])
